# revision 2
# baseline (speedup 1.0000x reference)
"""Trainium2 Bass kernel for nn_MockAttentionHead (metric-distance softmax attention).

Full inputs -> shard query rows across 8 cores, replicate keys -> full output.

Math (c = 0.01):
    s      = q . k                      (per query i, key j)
    u      = qq - s,  v = kk - s        (qq = |q|^2, kk = |k|^2)
    norm2  = u + v
    qd2    = norm2 + c*u^2  = c*P^2 + Q - 75   where P = u + 50, Q = v + 50
    kd2    = norm2 + c*v^2  = c*Q^2 + P - 75
    score  = -0.5*(sqrt(qd2) + sqrt(kd2))
    out    = softmax(score, axis=-1)

Device pipeline per [128 x 2048] tile:
    PE   : acc = -s                (4 f32r matmuls per 512-col bank)
    ACT  : X   = Square(sqc*acc + sqc*(qq+50))   = c*P^2
    DVE  : Qc  = (acc + 50) + kkrep              = Q        (scalar_tensor_tensor)
    ACT  : Y   = Square(sqc*Qc)                  = c*Q^2
    DVE  : T2  = (acc + (qq-25)) + Y   (in place on Y)
    DVE  : T1  = (Qc - 75) + X         (in place on Qc)
    ACT  : a = Sqrt(T1), b = Sqrt(T2)  (in place)
    DVE  : t = a + b -> row strip
Per [128 x 8192] row strip:
    ACT  : e = Exp(-0.5*t + 60), accum_out = rowsum   (single wide instr)
    DVE  : r = 1/rowsum ; strip *= r  ; DMA out
"""

import numpy as np

import concourse.bacc as bacc
import concourse.mybir as mybir
import concourse.tile as tile
from concourse.bass_utils import run_bass_kernel_spmd

N_CORES = 8
NQ = 8192
NK = 8192
D = 512
MQ = NQ // N_CORES          # query rows per core
M_TILES = MQ // 128         # 8
CHUNK = 2048
N_CHUNKS = NK // CHUNK      # 4
KC = D // 128               # 4 contraction chunks
SQC = 0.1                   # sqrt(METRIC_SCALE)
EXP_BIAS = 60.0

F32 = mybir.dt.float32
F32R = mybir.dt.float32r
AF = mybir.ActivationFunctionType
ALU = mybir.AluOpType


def build_nc():
    nc = bacc.Bacc(
        "TRN2", target_bir_lowering=False, debug=False, num_devices=N_CORES
    )
    # Exp bias constant (ACT float biases must exist as const APs).
    _cb = nc.alloc_sbuf_tensor(f"const-f32-{EXP_BIAS}", [128, 1], F32)
    nc.gpsimd.memset(_cb.ap(), EXP_BIAS)
    nc.const_aps.aps[(F32, EXP_BIAS)] = _cb.ap()
    nc.all_engine_barrier()

    qtn = nc.dram_tensor("qtn", [D, MQ], F32R, kind="ExternalInput")    # -q^T shard
    ktp = nc.dram_tensor("ktp", [D, NK], F32R, kind="ExternalInput")    # k^T (replicated)
    kkr = nc.dram_tensor("kkr", [128, NK], F32, kind="ExternalInput")   # kk bcast over partitions
    b1d = nc.dram_tensor("b1", [MQ, 1], F32, kind="ExternalInput")      # sqc*(qq+50)
    b2d = nc.dram_tensor("b2", [MQ, 1], F32, kind="ExternalInput")      # qq-25
    outd = nc.dram_tensor("out", [MQ, NK], F32, kind="ExternalOutput")

    ktv = ktp.ap().rearrange("(c p) n -> c p n", p=128)     # [KC, 128, NK]
    qtv = qtn.ap().rearrange("(c p) m -> p c m", p=128)     # [128, KC, MQ]

    with tile.TileContext(nc) as tc:
        with (
            tc.tile_pool(name="ktpool", bufs=1) as kt_pool,
            tc.tile_pool(name="qtpool", bufs=2) as qt_pool,
            tc.tile_pool(name="kkpool", bufs=2) as kk_pool,
            tc.tile_pool(name="workpool", bufs=4) as work_pool,
            tc.tile_pool(name="strippool", bufs=1) as strip_pool,
            tc.tile_pool(name="biaspool", bufs=2) as bias_pool,
            tc.tile_pool(name="rspool", bufs=2) as rs_pool,
            tc.tile_pool(name="psumpool", bufs=2, space="PSUM") as psum_pool,
        ):
            # resident k^T: 4 chunk tiles [128, NK]; split loads per 2048 cols
            kts = []
            for c in range(KC):
                ktc = kt_pool.tile([128, NK], F32R, name=f"kt{c}", tag=f"kt{c}")
                for b in range(N_CHUNKS):
                    nc.sync.dma_start(
                        out=ktc[:, b * CHUNK:(b + 1) * CHUNK],
                        in_=ktv[c, :, b * CHUNK:(b + 1) * CHUNK],
                    )
                kts.append(ktc)

            for m in range(M_TILES):
                ms = slice(m * 128, (m + 1) * 128)
                qtm = qt_pool.tile([128, KC * 128], F32R, name="qtm", tag="qtm")
                nc.sync.dma_start(
                    out=qtm.rearrange("p (c j) -> p c j", c=KC),
                    in_=qtv[:, :, ms],
                )
                b1t = bias_pool.tile([128, 1], F32, name="b1t", tag="b1t")
                nc.sync.dma_start(out=b1t, in_=b1d.ap()[ms, :])
                b2t = bias_pool.tile([128, 1], F32, name="b2t", tag="b2t")
                nc.sync.dma_start(out=b2t, in_=b2d.ap()[ms, :])

                strip = strip_pool.tile([128, NK], F32, name="strip", tag="strip")
                rs = rs_pool.tile([128, 1], F32, name="rs", tag="rs")

                for ch in range(N_CHUNKS):
                    n0 = ch * CHUNK
                    acc = psum_pool.tile([128, CHUNK], F32, name="acc", tag="acc")
                    for h in range(CHUNK // 512):
                        for c in range(KC):
                            nc.tensor.matmul(
                                acc[:, h * 512:(h + 1) * 512],
                                lhsT=qtm[:, c * 128:(c + 1) * 128],
                                rhs=kts[c][:, n0 + h * 512: n0 + (h + 1) * 512],
                                start=(c == 0),
                                stop=(c == KC - 1),
                            )

                    xc = work_pool.tile([128, CHUNK], F32, name="xc", tag="work")
                    qc = work_pool.tile([128, CHUNK], F32, name="qc", tag="work")
                    yc = work_pool.tile([128, CHUNK], F32, name="yc", tag="work")

                    # X = c*P^2
                    nc.scalar.activation(xc, acc, AF.Square, bias=b1t, scale=SQC)
                    # Qc = (acc + 50) + kk   (two 1024 halves, streamed kk tiles)
                    for hh in range(2):
                        hs = slice(hh * 1024, (hh + 1) * 1024)
                        kkt = kk_pool.tile([128, 1024], F32, name="kkt", tag="kkt")
                        nc.sync.dma_start(out=kkt, in_=kkr.ap()[:, n0 + hh * 1024: n0 + (hh + 1) * 1024])
                        nc.vector.scalar_tensor_tensor(
                            out=qc[:, hs], in0=acc[:, hs], scalar=50.0, in1=kkt,
                            op0=ALU.add, op1=ALU.add,
                        )
                    # Y = c*Q^2
                    nc.scalar.activation(yc, qc, AF.Square, scale=SQC)
                    # T2 = (acc + (qq-25)) + Y   in place on yc
                    nc.vector.scalar_tensor_tensor(
                        out=yc, in0=acc, scalar=b2t, in1=yc,
                        op0=ALU.add, op1=ALU.add,
                    )
                    # T1 = (Qc - 75) + X        in place on qc
                    nc.vector.scalar_tensor_tensor(
                        out=qc, in0=qc, scalar=-75.0, in1=xc,
                        op0=ALU.add, op1=ALU.add,
                    )
                    # a = sqrt(T1), b = sqrt(T2)
                    nc.scalar.activation(qc, qc, AF.Sqrt)
                    nc.scalar.activation(yc, yc, AF.Sqrt)
                    # t = a + b -> strip
                    nc.vector.tensor_add(strip[:, n0:n0 + CHUNK], qc, yc)

                # e = exp(-0.5*t + 60) in place, rowsum via accumulator
                nc.scalar.activation(
                    strip, strip, AF.Exp, bias=EXP_BIAS, scale=-0.5, accum_out=rs
                )
                rcp = rs_pool.tile([128, 1], F32, name="rcp", tag="rcp")
                nc.vector.reciprocal(rcp, rs)
                nc.vector.tensor_scalar(
                    out=strip, in0=strip, scalar1=rcp, scalar2=None, op0=ALU.mult
                )
                for b in range(N_CHUNKS):
                    nc.sync.dma_start(
                        out=outd.ap()[ms, b * CHUNK:(b + 1) * CHUNK],
                        in_=strip[:, b * CHUNK:(b + 1) * CHUNK],
                    )

    nc.compile()
    return nc


_NC_CACHE = None


def get_nc():
    global _NC_CACHE
    if _NC_CACHE is None:
        _NC_CACHE = build_nc()
    return _NC_CACHE


def make_in_maps(query_points: np.ndarray, key_points: np.ndarray):
    q = np.asarray(query_points, dtype=np.float32)
    k = np.asarray(key_points, dtype=np.float32)
    qq = np.sum(q * q, axis=1)
    kk = np.sum(k * k, axis=1)
    qtn_full = np.ascontiguousarray((-q).T)                # [D, NQ]
    ktp = np.ascontiguousarray(k.T)                        # [D, NK]
    kkr = np.ascontiguousarray(np.broadcast_to(kk, (128, NK)))
    b1_full = (SQC * (qq + 50.0)).astype(np.float32).reshape(-1, 1)
    b2_full = (qq - 25.0).astype(np.float32).reshape(-1, 1)

    in_maps = []
    for cid in range(N_CORES):
        sl = slice(cid * MQ, (cid + 1) * MQ)
        in_maps.append({
            "qtn": np.ascontiguousarray(qtn_full[:, sl]),
            "ktp": ktp,
            "kkr": kkr,
            "b1": np.ascontiguousarray(b1_full[sl]),
            "b2": np.ascontiguousarray(b2_full[sl]),
        })
    return in_maps


def kernel(query_points: np.ndarray, key_points: np.ndarray) -> np.ndarray:
    nc = get_nc()
    in_maps = make_in_maps(query_points, key_points)
    res = run_bass_kernel_spmd(nc, in_maps, core_ids=list(range(N_CORES)))
    out = np.concatenate(
        [res.results[c]["out"] for c in range(N_CORES)], axis=0
    )
    return out.astype(np.float32)


if __name__ == "__main__":
    rng = np.random.default_rng(0)
    q = rng.standard_normal((NQ, D), dtype=np.float32)
    k = rng.standard_normal((NK, D), dtype=np.float32)
    out = kernel(q, k)
    print(out.shape, out.dtype, out[0, :4])


# revision 4
# speedup vs baseline: 19596.2891x; 19596.2891x over previous
"""Trainium2 Bass kernel for nn_MockAttentionHead (metric-distance softmax attention).

Full inputs -> shard query rows across 8 cores, replicate keys -> full output.

Math (c = 0.01):
    s      = q . k                      (per query i, key j)
    u      = qq - s,  v = kk - s        (qq = |q|^2, kk = |k|^2)
    norm2  = u + v
    qd2    = norm2 + c*u^2  = c*P^2 + Q - 75   where P = u + 50, Q = v + 50
    kd2    = norm2 + c*v^2  = c*Q^2 + P - 75
    score  = -0.5*(sqrt(qd2) + sqrt(kd2))
    out    = softmax(score, axis=-1)

Device pipeline per [128 x 2048] tile:
    PE   : acc = -s                (f32r matmuls, or bf16 hi/lo x3 in precise mode)
    ACT  : X   = Square(sqc*acc + sqc*(qq+50))   = c*P^2
    DVE  : Qc  = (acc + 50) + kkrep              = Q        (scalar_tensor_tensor)
    ACT  : Y   = Square(sqc*Qc)                  = c*Q^2
    DVE  : T2  = (acc + (qq-25)) + Y   (in place on Y)
    DVE  : T1  = (Qc - 75) + X         (in place on Qc)
    ACT  : a = Sqrt(T1), b = Sqrt(T2)  (in place)
    DVE  : t = a + b -> row strip
Per [128 x 8192] row strip:
    ACT  : e = Exp(-0.5*t + 60), accum_out = rowsum   (single wide instr)
    DVE  : r = 1/rowsum ; strip *= r  ; DMA out
"""

import numpy as np
import ml_dtypes

import concourse.bacc as bacc
import concourse.mybir as mybir
import concourse.tile as tile
from concourse.bass_utils import run_bass_kernel_spmd

N_CORES = 8
NQ = 8192
NK = 8192
D = 512
MQ = NQ // N_CORES          # query rows per core
M_TILES = MQ // 128         # 8
CHUNK = 2048
N_CHUNKS = NK // CHUNK      # 4
KC = D // 128               # 4 contraction chunks
SQC = 0.1                   # sqrt(METRIC_SCALE)
EXP_BIAS = 60.0

PRECISE = True              # bf16 hi/lo x3 matmul (exact-ish) vs f32r single

F32 = mybir.dt.float32
F32R = mybir.dt.float32r
BF16 = mybir.dt.bfloat16
AF = mybir.ActivationFunctionType
ALU = mybir.AluOpType
BF16NP = ml_dtypes.bfloat16


def build_nc(precise: bool = PRECISE, repeat: int = 1):
    nc = bacc.Bacc(
        "TRN2", target_bir_lowering=False, debug=False, num_devices=N_CORES
    )
    # Exp bias constant (ACT float biases must exist as const APs).
    _cb = nc.alloc_sbuf_tensor(f"const-f32-{EXP_BIAS}", [128, 1], F32)
    nc.gpsimd.memset(_cb.ap(), EXP_BIAS)
    nc.const_aps.aps[(F32, EXP_BIAS)] = _cb.ap()
    nc.all_engine_barrier()

    if precise:
        qtn_h = nc.dram_tensor("qtn_h", [D, MQ], BF16, kind="ExternalInput")
        qtn_l = nc.dram_tensor("qtn_l", [D, MQ], BF16, kind="ExternalInput")
        ktp_h = nc.dram_tensor("ktp_h", [D, NK], BF16, kind="ExternalInput")
        ktp_l = nc.dram_tensor("ktp_l", [D, NK], BF16, kind="ExternalInput")
        q_drams = [qtn_h, qtn_l]
        k_drams = [ktp_h, ktp_l]
    else:
        qtn = nc.dram_tensor("qtn", [D, MQ], F32R, kind="ExternalInput")
        ktp = nc.dram_tensor("ktp", [D, NK], F32R, kind="ExternalInput")
        q_drams = [qtn]
        k_drams = [ktp]
    kkr = nc.dram_tensor("kkr", [128, NK], F32, kind="ExternalInput")
    b1d = nc.dram_tensor("b1", [MQ, 1], F32, kind="ExternalInput")
    b2d = nc.dram_tensor("b2", [MQ, 1], F32, kind="ExternalInput")
    outd = nc.dram_tensor("out", [MQ, NK], F32, kind="ExternalOutput")

    ktvs = [t.ap().rearrange("(c p) n -> c p n", p=128) for t in k_drams]
    qtvs = [t.ap().rearrange("(c p) m -> p c m", p=128) for t in q_drams]
    mm_dt = BF16 if precise else F32R

    with tile.TileContext(nc) as tc:
        with (
            tc.tile_pool(name="ktpool", bufs=1) as kt_pool,
            tc.tile_pool(name="qtpool", bufs=2) as qt_pool,
            tc.tile_pool(name="kkpool", bufs=2) as kk_pool,
            tc.tile_pool(name="workpool", bufs=4) as work_pool,
            tc.tile_pool(name="strippool", bufs=1) as strip_pool,
            tc.tile_pool(name="biaspool", bufs=2) as bias_pool,
            tc.tile_pool(name="rspool", bufs=2) as rs_pool,
            tc.tile_pool(name="psumpool", bufs=2, space="PSUM") as psum_pool,
        ):
            # resident k^T: per variant 4 chunk tiles [128, NK]
            kts = []  # kts[variant][c]
            for v, ktv in enumerate(ktvs):
                row = []
                for c in range(KC):
                    ktc = kt_pool.tile(
                        [128, NK], mm_dt, name=f"kt{v}_{c}", tag=f"kt{v}_{c}"
                    )
                    for b in range(N_CHUNKS):
                        nc.sync.dma_start(
                            out=ktc[:, b * CHUNK:(b + 1) * CHUNK],
                            in_=ktv[c, :, b * CHUNK:(b + 1) * CHUNK],
                        )
                    row.append(ktc)
                kts.append(row)

            import contextlib
            rep_ctx = (
                tc.For_i(0, repeat, 1) if repeat > 1 else contextlib.nullcontext()
            )
            with rep_ctx:
                for m in range(M_TILES):
                    ms = slice(m * 128, (m + 1) * 128)
                    qtms = []
                    for v, qtv in enumerate(qtvs):
                        qtm = qt_pool.tile(
                            [128, KC * 128], mm_dt, name=f"qtm{v}", tag=f"qtm{v}"
                        )
                        nc.sync.dma_start(
                            out=qtm.rearrange("p (c j) -> p c j", c=KC),
                            in_=qtv[:, :, ms],
                        )
                        qtms.append(qtm)
                    b1t = bias_pool.tile([128, 1], F32, name="b1t", tag="b1t")
                    nc.sync.dma_start(out=b1t, in_=b1d.ap()[ms, :])
                    b2t = bias_pool.tile([128, 1], F32, name="b2t", tag="b2t")
                    nc.sync.dma_start(out=b2t, in_=b2d.ap()[ms, :])

                    strip = strip_pool.tile([128, NK], F32, name="strip", tag="strip")
                    rs = rs_pool.tile([128, 1], F32, name="rs", tag="rs")

                    # operand pairs for the matmul accumulation chain
                    if precise:
                        pairs = [(0, 0), (0, 1), (1, 0)]  # qh*kh + qh*kl + ql*kh
                    else:
                        pairs = [(0, 0)]

                    for ch in range(N_CHUNKS):
                        n0 = ch * CHUNK
                        acc = psum_pool.tile([128, CHUNK], F32, name="acc", tag="acc")
                        for h in range(CHUNK // 512):
                            first = True
                            for (qv, kv) in pairs:
                                for c in range(KC):
                                    nc.tensor.matmul(
                                        acc[:, h * 512:(h + 1) * 512],
                                        lhsT=qtms[qv][:, c * 128:(c + 1) * 128],
                                        rhs=kts[kv][c][:, n0 + h * 512: n0 + (h + 1) * 512],
                                        start=first,
                                        stop=(qv, kv) == pairs[-1] and c == KC - 1,
                                    )
                                    first = False

                        xc = work_pool.tile([128, CHUNK], F32, name="xc", tag="work")
                        qc = work_pool.tile([128, CHUNK], F32, name="qc", tag="work")
                        yc = work_pool.tile([128, CHUNK], F32, name="yc", tag="work")

                        # X = c*P^2
                        nc.scalar.activation(xc, acc, AF.Square, bias=b1t, scale=SQC)
                        # Qc = (acc + 50) + kk   (two 1024 halves, streamed kk tiles)
                        for hh in range(2):
                            hs = slice(hh * 1024, (hh + 1) * 1024)
                            kkt = kk_pool.tile([128, 1024], F32, name="kkt", tag="kkt")
                            nc.sync.dma_start(
                                out=kkt,
                                in_=kkr.ap()[:, n0 + hh * 1024: n0 + (hh + 1) * 1024],
                            )
                            nc.vector.scalar_tensor_tensor(
                                out=qc[:, hs], in0=acc[:, hs], scalar=50.0, in1=kkt,
                                op0=ALU.add, op1=ALU.add,
                            )
                        # Y = c*Q^2
                        nc.scalar.activation(yc, qc, AF.Square, scale=SQC)
                        # T2 = (acc + (qq-25)) + Y   in place on yc
                        nc.vector.scalar_tensor_tensor(
                            out=yc, in0=acc, scalar=b2t, in1=yc,
                            op0=ALU.add, op1=ALU.add,
                        )
                        # T1 = (Qc - 75) + X        in place on qc
                        nc.vector.scalar_tensor_tensor(
                            out=qc, in0=qc, scalar=-75.0, in1=xc,
                            op0=ALU.add, op1=ALU.add,
                        )
                        # a = sqrt(T1), b = sqrt(T2)
                        nc.scalar.activation(qc, qc, AF.Sqrt)
                        nc.scalar.activation(yc, yc, AF.Sqrt)
                        # t = a + b -> strip
                        nc.vector.tensor_add(strip[:, n0:n0 + CHUNK], qc, yc)

                    # e = exp(-0.5*t + 60) in place, rowsum via accumulator
                    nc.scalar.activation(
                        strip, strip, AF.Exp, bias=EXP_BIAS, scale=-0.5, accum_out=rs
                    )
                    rcp = rs_pool.tile([128, 1], F32, name="rcp", tag="rcp")
                    nc.vector.reciprocal(rcp, rs)
                    nc.vector.tensor_scalar(
                        out=strip, in0=strip, scalar1=rcp, scalar2=None, op0=ALU.mult
                    )
                    for b in range(N_CHUNKS):
                        nc.sync.dma_start(
                            out=outd.ap()[ms, b * CHUNK:(b + 1) * CHUNK],
                            in_=strip[:, b * CHUNK:(b + 1) * CHUNK],
                        )

    nc.compile()
    return nc


_NC_CACHE = {}


def get_nc(precise: bool = PRECISE, repeat: int = 1):
    key = (precise, repeat)
    if key not in _NC_CACHE:
        _NC_CACHE[key] = build_nc(precise, repeat)
    return _NC_CACHE[key]


def make_in_maps(query_points: np.ndarray, key_points: np.ndarray,
                 precise: bool = PRECISE):
    q = np.asarray(query_points, dtype=np.float32)
    k = np.asarray(key_points, dtype=np.float32)
    qq = np.sum(q * q, axis=1)
    kk = np.sum(k * k, axis=1)
    qtn_full = np.ascontiguousarray((-q).T)                # [D, NQ]
    ktp = np.ascontiguousarray(k.T)                        # [D, NK]
    kkr = np.ascontiguousarray(np.broadcast_to(kk, (128, NK)))
    b1_full = (SQC * (qq + 50.0)).astype(np.float32).reshape(-1, 1)
    b2_full = (qq - 25.0).astype(np.float32).reshape(-1, 1)

    if precise:
        ktp_h = ktp.astype(BF16NP)
        ktp_l = (ktp - ktp_h.astype(np.float32)).astype(BF16NP)
        qtn_h_full = qtn_full.astype(BF16NP)
        qtn_l_full = (qtn_full - qtn_h_full.astype(np.float32)).astype(BF16NP)

    in_maps = []
    for cid in range(N_CORES):
        sl = slice(cid * MQ, (cid + 1) * MQ)
        m = {
            "kkr": kkr,
            "b1": np.ascontiguousarray(b1_full[sl]),
            "b2": np.ascontiguousarray(b2_full[sl]),
        }
        if precise:
            m["qtn_h"] = np.ascontiguousarray(qtn_h_full[:, sl])
            m["qtn_l"] = np.ascontiguousarray(qtn_l_full[:, sl])
            m["ktp_h"] = ktp_h
            m["ktp_l"] = ktp_l
        else:
            m["qtn"] = np.ascontiguousarray(qtn_full[:, sl])
            m["ktp"] = ktp
        in_maps.append(m)
    return in_maps


def kernel(query_points: np.ndarray, key_points: np.ndarray) -> np.ndarray:
    nc = get_nc()
    in_maps = make_in_maps(query_points, key_points)
    res = run_bass_kernel_spmd(nc, in_maps, core_ids=list(range(N_CORES)))
    out = np.concatenate(
        [res.results[c]["out"] for c in range(N_CORES)], axis=0
    )
    return out.astype(np.float32)


if __name__ == "__main__":
    rng = np.random.default_rng(0)
    q = rng.standard_normal((NQ, D), dtype=np.float32)
    k = rng.standard_normal((NK, D), dtype=np.float32)
    out = kernel(q, k)
    print(out.shape, out.dtype, out[0, :4])


# revision 25
# speedup vs baseline: 19715.4870x; 1.0061x over previous
"""Trainium2 Bass kernel for nn_MockAttentionHead (metric-distance softmax attention).

Full inputs -> shard query rows across 8 cores, replicate keys -> full output.

Math (c = 0.01):
    s      = q . k                      (per query i, key j)
    u      = qq - s,  v = kk - s        (qq = |q|^2, kk = |k|^2)
    norm2  = u + v
    qd2    = norm2 + c*u^2  = c*P^2 + Q - 75   where P = u + 50, Q = v + 50
    kd2    = norm2 + c*v^2  = c*Q^2 + P - 75
    score  = -0.5*(sqrt(qd2) + sqrt(kd2))
    out    = softmax(score, axis=-1)

Device pipeline per [128 x 2048] tile:
    PE   : acc = -s                (f32r matmuls, or bf16 hi/lo x3 in precise mode)
    ACT  : X   = Square(sqc*acc + sqc*(qq+50))   = c*P^2
    DVE  : Qc  = (acc + 50) + kkrep              = Q        (scalar_tensor_tensor)
    ACT  : Y   = Square(sqc*Qc)                  = c*Q^2
    DVE  : T2  = (acc + (qq-25)) + Y   (in place on Y)
    DVE  : T1  = (Qc - 75) + X         (in place on Qc)
    ACT  : a = Sqrt(T1), b = Sqrt(T2)  (in place)
    DVE  : t = a + b -> row strip
Per [128 x 8192] row strip:
    ACT  : e = Exp(-0.5*t + 60), accum_out = rowsum   (single wide instr)
    DVE  : r = 1/rowsum ; strip *= r  ; DMA out
"""

import numpy as np
import ml_dtypes

import concourse.bacc as bacc
import concourse.mybir as mybir
import concourse.tile as tile
from concourse.bass_utils import run_bass_kernel_spmd

N_CORES = 8
NQ = 8192
NK = 8192
D = 512
MQ = NQ // N_CORES          # query rows per core
M_TILES = MQ // 128         # 8
CHUNK = 2048
N_CHUNKS = NK // CHUNK      # 4
KC = D // 128               # 4 contraction chunks
SQC = 0.1                   # sqrt(METRIC_SCALE)
EXP_BIAS = 60.0

PRECISE = True              # bf16 hi/lo x3 matmul (exact-ish) vs f32r single

# optimization knobs (module-level so bench variants can flip them)
OPT_POOL_TADD = False       # t = a + b on GpSimd instead of DVE
OPT_WIDE_KK = False         # kk chunks at 2048 (single stt) instead of 2x1024
OPT_WEIGHT_REUSE = False    # order matmuls so consecutive MMs share lhsT
OPT_SPLIT_NORM = True       # normalize per 2048-chunk (early strip release)
OPT_DMA_ORDER = True        # first-needed kT blocks DMA'd first
OPT_WORK_BUFS = 4           # work pool slots (3 live per chunk)
OPT_PREFETCH = True         # input loads on gpsimd queue, one m-tile ahead
OPT_SPLIT_EXP = False       # exp per chunk (risks ACT table thrash)
OPT_DEFER_EPI = False       # emit epilogue after next tile's first chunk

F32 = mybir.dt.float32
F32R = mybir.dt.float32r
BF16 = mybir.dt.bfloat16
AF = mybir.ActivationFunctionType
ALU = mybir.AluOpType
BF16NP = ml_dtypes.bfloat16


def build_nc(precise: bool = PRECISE, repeat: int = 1):
    nc = bacc.Bacc(
        "TRN2", target_bir_lowering=False, debug=False, num_devices=N_CORES
    )
    # Exp bias constant (ACT float biases must exist as const APs).
    _cb = nc.alloc_sbuf_tensor(f"const-f32-{EXP_BIAS}", [128, 1], F32)
    nc.gpsimd.memset(_cb.ap(), EXP_BIAS)
    nc.const_aps.aps[(F32, EXP_BIAS)] = _cb.ap()
    nc.all_engine_barrier()

    if precise:
        qtn_h = nc.dram_tensor("qtn_h", [D, MQ], BF16, kind="ExternalInput")
        qtn_l = nc.dram_tensor("qtn_l", [D, MQ], BF16, kind="ExternalInput")
        ktp_h = nc.dram_tensor("ktp_h", [D, NK], BF16, kind="ExternalInput")
        ktp_l = nc.dram_tensor("ktp_l", [D, NK], BF16, kind="ExternalInput")
        q_drams = [qtn_h, qtn_l]
        k_drams = [ktp_h, ktp_l]
    else:
        qtn = nc.dram_tensor("qtn", [D, MQ], F32R, kind="ExternalInput")
        ktp = nc.dram_tensor("ktp", [D, NK], F32R, kind="ExternalInput")
        q_drams = [qtn]
        k_drams = [ktp]
    kkr = nc.dram_tensor("kkr", [128, NK], F32, kind="ExternalInput")
    b1d = nc.dram_tensor("b1", [MQ, 1], F32, kind="ExternalInput")
    b2d = nc.dram_tensor("b2", [MQ, 1], F32, kind="ExternalInput")
    outd = nc.dram_tensor("out", [MQ, NK], F32, kind="ExternalOutput")

    ktvs = [t.ap().rearrange("(c p) n -> c p n", p=128) for t in k_drams]
    qtvs = [t.ap().rearrange("(c p) m -> p c m", p=128) for t in q_drams]
    mm_dt = BF16 if precise else F32R

    with tile.TileContext(nc) as tc:
        with (
            tc.tile_pool(name="ktpool", bufs=1) as kt_pool,
            tc.tile_pool(name="qtpool", bufs=1 if OPT_WORK_BUFS >= 5 else 2) as qt_pool,
            tc.tile_pool(name="kkpool", bufs=1 if OPT_WORK_BUFS >= 5 else 2) as kk_pool,
            tc.tile_pool(name="workpool", bufs=OPT_WORK_BUFS) as work_pool,
            tc.tile_pool(name="strippool", bufs=1) as strip_pool,
            tc.tile_pool(name="biaspool", bufs=2) as bias_pool,
            tc.tile_pool(name="rspool", bufs=2) as rs_pool,
            tc.tile_pool(name="psumpool", bufs=2, space="PSUM") as psum_pool,
        ):
            # engine whose queue carries input prefetches (POOL's SWDGE is idle;
            # SP carries the output DMAs and would delay these)
            in_eng = nc.gpsimd if OPT_PREFETCH else nc.sync

            def load_mtile_inputs(m):
                """Allocate + DMA qtm/b1/b2 for m-tile m on the prefetch queue."""
                ms_ = slice(m * 128, (m + 1) * 128)
                qtms_ = []
                for v, qtv in enumerate(qtvs):
                    qtm = qt_pool.tile(
                        [128, KC * 128], mm_dt, name=f"qtm{v}", tag=f"qtm{v}"
                    )
                    in_eng.dma_start(
                        out=qtm.rearrange("p (c j) -> p c j", c=KC),
                        in_=qtv[:, :, ms_],
                    )
                    qtms_.append(qtm)
                b1t_ = bias_pool.tile([128, 1], F32, name="b1t", tag="b1t")
                in_eng.dma_start(out=b1t_, in_=b1d.ap()[ms_, :])
                b2t_ = bias_pool.tile([128, 1], F32, name="b2t", tag="b2t")
                in_eng.dma_start(out=b2t_, in_=b2d.ap()[ms_, :])
                return qtms_, b1t_, b2t_

            # resident k^T: per variant 4 chunk tiles [128, NK]
            kts = []  # kts[variant][c]
            for v, ktv in enumerate(ktvs):
                row = []
                for c in range(KC):
                    ktc = kt_pool.tile(
                        [128, NK], mm_dt, name=f"kt{v}_{c}", tag=f"kt{v}_{c}"
                    )
                    row.append(ktc)
                kts.append(row)

            # first m-tile inputs land before the big kT transfers
            pending = load_mtile_inputs(0) if OPT_PREFETCH else None

            if OPT_DMA_ORDER:
                # column-block-major (first-needed first), alternating between
                # the two HWDGE queues (SP / ACT)
                qi = 0
                for b in range(N_CHUNKS):
                    for v, ktv in enumerate(ktvs):
                        for c in range(KC):
                            eng = nc.sync if qi % 2 == 0 else nc.scalar
                            qi += 1
                            eng.dma_start(
                                out=kts[v][c][:, b * CHUNK:(b + 1) * CHUNK],
                                in_=ktv[c, :, b * CHUNK:(b + 1) * CHUNK],
                            )
            else:
                for v, ktv in enumerate(ktvs):
                    for c in range(KC):
                        for b in range(N_CHUNKS):
                            nc.sync.dma_start(
                                out=kts[v][c][:, b * CHUNK:(b + 1) * CHUNK],
                                in_=ktv[c, :, b * CHUNK:(b + 1) * CHUNK],
                            )

            def emit_epilogue(strip, rs, ms):
                """exp + rowsum + normalize + store for one finished m-tile.

                Exp is split per chunk (Square is in every ACT table set, so
                the scheduler may interleave the next tile's squares without
                a table switch). Emission is deferred one m-tile so the next
                tile's early chunk work can be scheduled ahead of it.
                """
                if OPT_SPLIT_EXP:
                    for b in range(N_CHUNKS):
                        sl = slice(b * CHUNK, (b + 1) * CHUNK)
                        nc.scalar.activation(
                            strip[:, sl], strip[:, sl], AF.Exp,
                            bias=EXP_BIAS, scale=-0.5, accum_out=rs[:, b:b + 1],
                        )
                    rst = rs_pool.tile([128, 1], F32, name="rst", tag="rst")
                    nc.vector.tensor_reduce(
                        rst, rs, axis=mybir.AxisListType.X, op=ALU.add
                    )
                else:
                    nc.scalar.activation(
                        strip, strip, AF.Exp,
                        bias=EXP_BIAS, scale=-0.5, accum_out=rs[:, 0:1],
                    )
                    rst = rs[:, 0:1]
                rcp = rs_pool.tile([128, 1], F32, name="rcp", tag="rcp")
                nc.vector.reciprocal(rcp, rst)
                if OPT_SPLIT_NORM:
                    for b in range(N_CHUNKS):
                        sl = slice(b * CHUNK, (b + 1) * CHUNK)
                        nc.vector.tensor_scalar(
                            out=strip[:, sl], in0=strip[:, sl],
                            scalar1=rcp, scalar2=None, op0=ALU.mult,
                        )
                        nc.sync.dma_start(out=outd.ap()[ms, sl], in_=strip[:, sl])
                else:
                    nc.vector.tensor_scalar(
                        out=strip, in0=strip, scalar1=rcp, scalar2=None,
                        op0=ALU.mult,
                    )
                    for b in range(N_CHUNKS):
                        sl = slice(b * CHUNK, (b + 1) * CHUNK)
                        nc.sync.dma_start(out=outd.ap()[ms, sl], in_=strip[:, sl])

            import contextlib
            rep_ctx = (
                tc.For_i(0, repeat, 1) if repeat > 1 else contextlib.nullcontext()
            )
            with rep_ctx:
                pend_epi = None
                for m in range(M_TILES):
                    ms = slice(m * 128, (m + 1) * 128)
                    if pending is not None:
                        qtms, b1t, b2t = pending
                        # prefetch next m-tile (wraps for the repeat loop)
                        pending = load_mtile_inputs((m + 1) % M_TILES)
                    else:
                        qtms, b1t, b2t = load_mtile_inputs(m)

                    strip = strip_pool.tile([128, NK], F32, name="strip", tag="strip")
                    rs = rs_pool.tile([128, N_CHUNKS], F32, name="rs", tag="rs")

                    # operand pairs for the matmul accumulation chain
                    if precise:
                        pairs = [(0, 0), (0, 1), (1, 0)]  # qh*kh + qh*kl + ql*kh
                    else:
                        pairs = [(0, 0)]

                    for ch in range(N_CHUNKS):
                        n0 = ch * CHUNK
                        acc = psum_pool.tile([128, CHUNK], F32, name="acc", tag="acc")
                        if OPT_WEIGHT_REUSE:
                            # outer loop over (pair, k-chunk): 4 consecutive MMs
                            # share the same stationary operand
                            np_pairs = len(pairs)
                            for pi, (qv, kv) in enumerate(pairs):
                                for c in range(KC):
                                    for h in range(CHUNK // 512):
                                        nc.tensor.matmul(
                                            acc[:, h * 512:(h + 1) * 512],
                                            lhsT=qtms[qv][:, c * 128:(c + 1) * 128],
                                            rhs=kts[kv][c][:, n0 + h * 512: n0 + (h + 1) * 512],
                                            start=(pi == 0 and c == 0),
                                            stop=(pi == np_pairs - 1 and c == KC - 1),
                                        )
                        else:
                            for h in range(CHUNK // 512):
                                first = True
                                for (qv, kv) in pairs:
                                    for c in range(KC):
                                        nc.tensor.matmul(
                                            acc[:, h * 512:(h + 1) * 512],
                                            lhsT=qtms[qv][:, c * 128:(c + 1) * 128],
                                            rhs=kts[kv][c][:, n0 + h * 512: n0 + (h + 1) * 512],
                                            start=first,
                                            stop=(qv, kv) == pairs[-1] and c == KC - 1,
                                        )
                                        first = False

                        xc = work_pool.tile([128, CHUNK], F32, name="xc", tag="work")
                        qc = work_pool.tile([128, CHUNK], F32, name="qc", tag="work")
                        yc = work_pool.tile([128, CHUNK], F32, name="yc", tag="work")

                        # X = c*P^2
                        nc.scalar.activation(xc, acc, AF.Square, bias=b1t, scale=SQC)
                        # Qc = (acc + 50) + kk   (streamed kk tiles)
                        kk_w = CHUNK if OPT_WIDE_KK else 1024
                        for hh in range(CHUNK // kk_w):
                            hs = slice(hh * kk_w, (hh + 1) * kk_w)
                            kkt = kk_pool.tile([128, kk_w], F32, name="kkt", tag="kkt")
                            in_eng.dma_start(
                                out=kkt,
                                in_=kkr.ap()[:, n0 + hh * kk_w: n0 + (hh + 1) * kk_w],
                            )
                            nc.vector.scalar_tensor_tensor(
                                out=qc[:, hs], in0=acc[:, hs], scalar=50.0, in1=kkt,
                                op0=ALU.add, op1=ALU.add,
                            )
                        # Y = c*Q^2
                        nc.scalar.activation(yc, qc, AF.Square, scale=SQC)
                        # T2 = (acc + (qq-25)) + Y   in place on yc
                        nc.vector.scalar_tensor_tensor(
                            out=yc, in0=acc, scalar=b2t, in1=yc,
                            op0=ALU.add, op1=ALU.add,
                        )
                        # T1 = (Qc - 75) + X        in place on qc
                        nc.vector.scalar_tensor_tensor(
                            out=qc, in0=qc, scalar=-75.0, in1=xc,
                            op0=ALU.add, op1=ALU.add,
                        )
                        # a = sqrt(T1), b = sqrt(T2)
                        nc.scalar.activation(qc, qc, AF.Sqrt)
                        nc.scalar.activation(yc, yc, AF.Sqrt)
                        # t = a + b -> strip
                        if OPT_POOL_TADD:
                            nc.gpsimd.tensor_add(strip[:, n0:n0 + CHUNK], qc, yc)
                        else:
                            nc.vector.tensor_add(strip[:, n0:n0 + CHUNK], qc, yc)

                        # previous m-tile's epilogue lands after our first
                        # chunk so its exp doesn't starve this tile's chain
                        if OPT_DEFER_EPI and ch == 0 and pend_epi is not None:
                            emit_epilogue(*pend_epi)
                            pend_epi = None

                    if OPT_DEFER_EPI:
                        pend_epi = (strip, rs, ms)
                    else:
                        emit_epilogue(strip, rs, ms)
                if pend_epi is not None:
                    emit_epilogue(*pend_epi)
                    pend_epi = None

    nc.compile()
    return nc


_NC_CACHE = {}


def get_nc(precise: bool = PRECISE, repeat: int = 1):
    key = (precise, repeat, OPT_POOL_TADD, OPT_WIDE_KK, OPT_WEIGHT_REUSE,
           OPT_SPLIT_NORM, OPT_DMA_ORDER, OPT_WORK_BUFS, OPT_PREFETCH,
           OPT_SPLIT_EXP, OPT_DEFER_EPI)
    if key not in _NC_CACHE:
        _NC_CACHE[key] = build_nc(precise, repeat)
    return _NC_CACHE[key]


def make_in_maps(query_points: np.ndarray, key_points: np.ndarray,
                 precise: bool = PRECISE):
    q = np.asarray(query_points, dtype=np.float32)
    k = np.asarray(key_points, dtype=np.float32)
    qq = np.sum(q * q, axis=1)
    kk = np.sum(k * k, axis=1)
    qtn_full = np.ascontiguousarray((-q).T)                # [D, NQ]
    ktp = np.ascontiguousarray(k.T)                        # [D, NK]
    kkr = np.ascontiguousarray(np.broadcast_to(kk, (128, NK)))
    b1_full = (SQC * (qq + 50.0)).astype(np.float32).reshape(-1, 1)
    b2_full = (qq - 25.0).astype(np.float32).reshape(-1, 1)

    if precise:
        ktp_h = ktp.astype(BF16NP)
        ktp_l = (ktp - ktp_h.astype(np.float32)).astype(BF16NP)
        qtn_h_full = qtn_full.astype(BF16NP)
        qtn_l_full = (qtn_full - qtn_h_full.astype(np.float32)).astype(BF16NP)

    in_maps = []
    for cid in range(N_CORES):
        sl = slice(cid * MQ, (cid + 1) * MQ)
        m = {
            "kkr": kkr,
            "b1": np.ascontiguousarray(b1_full[sl]),
            "b2": np.ascontiguousarray(b2_full[sl]),
        }
        if precise:
            m["qtn_h"] = np.ascontiguousarray(qtn_h_full[:, sl])
            m["qtn_l"] = np.ascontiguousarray(qtn_l_full[:, sl])
            m["ktp_h"] = ktp_h
            m["ktp_l"] = ktp_l
        else:
            m["qtn"] = np.ascontiguousarray(qtn_full[:, sl])
            m["ktp"] = ktp
        in_maps.append(m)
    return in_maps


def kernel(query_points: np.ndarray, key_points: np.ndarray) -> np.ndarray:
    nc = get_nc()
    in_maps = make_in_maps(query_points, key_points)
    res = run_bass_kernel_spmd(nc, in_maps, core_ids=list(range(N_CORES)))
    out = np.concatenate(
        [res.results[c]["out"] for c in range(N_CORES)], axis=0
    )
    return out.astype(np.float32)


if __name__ == "__main__":
    rng = np.random.default_rng(0)
    q = rng.standard_normal((NQ, D), dtype=np.float32)
    k = rng.standard_normal((NK, D), dtype=np.float32)
    out = kernel(q, k)
    print(out.shape, out.dtype, out[0, :4])
